# revision 34
# baseline (speedup 1.0000x reference)
"""MoE (top-1 routed) Trainium2 kernel — v3.

Routing on host (bitwise-matching the reference's fp32 `x @ Wg + bg`
argmax on CPU); expert e's tokens run on NeuronCore e (expert-parallel,
all-reduce-free).  Device math per core, transposed layout (features on
partitions, tokens on free dim), t2 = tanh(z/2), xn = (1+t2)/2,
k_j = j/7, b_j = exp(-32 (xn-k_j)^2):

    h^T  = W1^T x^T                  (PE bf16, K=1024)
    sw   = (tanh(h/2) + 1) * h       == 2*swish(h)     (ACT+DVE)
    z^T  = (0.5*proj)^T sw           (PE bf16)
    q^j  = exp(esc_j*(t2+1)), esc_j = 32j/7   (odd j=1,3,5 on ACT;
           q2=q1^2, q4=q1*q3, q6=q3^2, q7=q3*q4 as bf16 DVE mults)
    F    = exp(-8 (1+t2)^2) == b_0   (ACT square + exp)
    num  = cv0' + sum_j cvj' q^j     (PE diag-bf16 matmuls, cv0 via a
                                      ones-stream; one vc chunk on DVE)
    out  = F * num                   (gpsimd)

cvj' = ctrl_j*scaling*a_j/theta with a_j = exp(-32 k_j^2).  The RBF
normalizer sum_j b_j is ~= theta = 2.193299 (theta-function flatness,
5.3e-3 worst for observed xn in [0.27,0.77]) and is folded into cvj',
removing the denominator + reciprocal entirely.

All elementwise ops run full-width [128, C] (C~1152) in ONE instruction
to amortize the fixed ~230ns/instr engine overheads: PSUM h/z tiles are
multi-bank mega-tiles; matmuls write 512-aligned in-bank slices.
PSUM budget: 2 mega slots (3 banks each) + 2 single-bank num slots = 8.
"""

import os
from contextlib import ExitStack

import numpy as np

N_TOK, D_IN, U_DIM, E_EXP, B_BAS = 8192, 1024, 512, 8, 8
N_CORES = 8
P = 128
THETA = 2.1932985352029515

MM_MODE = os.environ.get("MOE_MM_MODE", "bf16")
DVE_VC = int(os.environ.get("MOE_DVE_VC", "0"))  # vc chunk whose num runs on DVE

_prog_cache = {}


def _basis_consts():
    ks = np.linspace(0.0, 1.0, B_BAS).astype(np.float64)
    a = np.exp(-32.0 * ks * ks)
    esc = 32.0 * ks * 7.0 / 7.0  # 32*j/7
    return ks, a, esc


def build_program(C, b1_zero):
    import concourse.tile as tile
    from concourse import bacc, mybir

    f32 = mybir.dt.float32
    bf16 = mybir.dt.bfloat16
    add = mybir.AluOpType.add
    mult = mybir.AluOpType.mult
    Tanh = mybir.ActivationFunctionType.Tanh
    Exp = mybir.ActivationFunctionType.Exp
    Square = mybir.ActivationFunctionType.Square

    assert C % 128 == 0
    # 512-wide bank-aligned chunks (the matmul write granularity)
    chunks = []
    t0 = 0
    while t0 < C:
        chunks.append((t0, min(512, C - t0)))
        t0 += 512

    _, _, esc = _basis_consts()

    nc = bacc.Bacc("TRN2", target_bir_lowering=False, debug=False,
                   num_devices=N_CORES)

    xT = nc.dram_tensor("xT", [D_IN, C], bf16, kind="ExternalInput").ap()
    w1 = nc.dram_tensor("w1", [4, P, 8 * P], bf16, kind="ExternalInput").ap()
    p5 = nc.dram_tensor("p5", [U_DIM, U_DIM], bf16, kind="ExternalInput").ap()
    aux = nc.dram_tensor("aux", [P, 32, P], bf16, kind="ExternalInput").ap()
    cv0 = nc.dram_tensor("cv0", [P, 4], f32, kind="ExternalInput").ap()
    cvj = nc.dram_tensor("cvj", [P, 4, B_BAS], f32, kind="ExternalInput").ap()
    b1h = nc.dram_tensor("b1h", [P, 4], f32, kind="ExternalInput").ap()
    outT = nc.dram_tensor("outT", [U_DIM, C], f32, kind="ExternalOutput").ap()

    xT_r = xT.rearrange("(kc p) c -> p kc c", p=P)          # [128, 8, C]
    w1_r = w1.rearrange("u p k -> p u k")                   # [128, 4, 1024]
    p5_r = p5.rearrange("(uc p) v -> p uc v", p=P)          # [128, 4, 512]
    outT_r = outT.rearrange("(vc p) c -> p vc c", p=P)      # [128, 4, C]

    with tile.TileContext(nc) as tc, ExitStack() as ctx:
        cpool = ctx.enter_context(tc.tile_pool(name="consts", bufs=1))
        bigps = ctx.enter_context(tc.tile_pool(name="bigps", bufs=2,
                                               space="PSUM"))
        npps = ctx.enter_context(tc.tile_pool(name="npps", bufs=2,
                                              space="PSUM"))
        wpool = ctx.enter_context(tc.tile_pool(name="work", bufs=2))
        swpool = ctx.enter_context(tc.tile_pool(name="sw", bufs=4))
        gpool = ctx.enter_context(tc.tile_pool(name="g", bufs=14))

        # ---- input DMA ----
        w1u = []
        for uc in range(4):
            t = cpool.tile([P, 8 * P], bf16, tag=f"w1_{uc}")
            w1u.append(t)
        xk = [cpool.tile([P, C], bf16, tag=f"x{kc}", name=f"x{kc}")
              for kc in range(8)]
        # x + w1 race in first on sync+scalar, arrival ~ consumption order;
        # late-needed bulk (p5, aux, cvj) queues behind on sync.  The very
        # first 512-col chunk of xk0 and kc0 of w1u0 ship separately so the
        # first matmul can start ~2us earlier.
        # ones memset on the idle DVE queue so the PE warmup starts ASAP
        ones = cpool.tile([P, 512], bf16, tag="ones")
        nc.vector.memset(ones[:], 1.0)
        nc.sync.dma_start(xk[0][:, 0:512], xT_r[:, 0, 0:512])
        nc.scalar.dma_start(w1u[0][:, 0:P], w1_r[:, 0, 0:P])
        nc.sync.dma_start(xk[0][:, 512:C], xT_r[:, 0, 512:C])
        nc.scalar.dma_start(xk[1][:], xT_r[:, 1, :])
        nc.scalar.dma_start(w1u[0][:, P:8 * P], w1_r[:, 0, P:8 * P])
        nc.sync.dma_start(xk[3][:], xT_r[:, 3, :])
        nc.scalar.dma_start(xk[2][:], xT_r[:, 2, :])
        nc.scalar.dma_start(w1u[1][:], w1_r[:, 1, :])
        nc.scalar.dma_start(xk[4][:], xT_r[:, 4, :])
        nc.scalar.dma_start(xk[6][:], xT_r[:, 6, :])
        nc.scalar.dma_start(w1u[2][:], w1_r[:, 2, :])
        nc.sync.dma_start(w1u[3][:], w1_r[:, 3, :])
        nc.gpsimd.dma_start(xk[5][:], xT_r[:, 5, :])
        nc.gpsimd.dma_start(xk[7][:], xT_r[:, 7, :])
        puc = []
        for uc in range(4):
            t = cpool.tile([P, U_DIM], bf16, tag=f"p5_{uc}")
            q = nc.sync if uc % 2 == 0 else nc.scalar
            q.dma_start(t[:], p5_r[:, uc, :])
            puc.append(t)
        cv0sb = cpool.tile([P, 4], f32, tag="cv0")
        nc.sync.dma_start(cv0sb[:], cv0[:])
        cvjsb = cpool.tile([P, 4, B_BAS], f32, tag="cvj")
        nc.sync.dma_start(cvjsb[:], cvj[:])
        auxsb = cpool.tile([P, 32, P], bf16, tag="aux")
        nc.scalar.dma_start(auxsb[:], aux[:])
        if not b1_zero:
            b1sb = cpool.tile([P, 4], f32, tag="b1h")
            nc.sync.dma_start(b1sb[:], b1h[:])
        # bias constants for ACT + bf16 ones (rhs for the cv0 stream)
        bias_vals = [float(esc[1]), float(esc[3]), float(esc[5]), 1.0]
        bsb = cpool.tile([P, len(bias_vals)], f32, tag="bias")
        for i, v in enumerate(bias_vals):
            nc.gpsimd.memset(bsb[:, i:i + 1], v)
        bias_of = {1: bsb[:, 0:1], 3: bsb[:, 1:2], 5: bsb[:, 2:3]}
        one_b = bsb[:, 3:4]

        # ---- PE warmup: ramp the tensor-engine p-state to max while the
        # first x/w1 DMAs are in flight (dummy matmuls on the ones tile) ----
        N_WARM = int(os.environ.get("MOE_WARM", "16"))
        if N_WARM:
            wps = npps.tile([P, 512], f32, tag="np", name="warm")
            for i in range(N_WARM):
                nc.tensor.matmul(wps[:], lhsT=ones[:, 0:P], rhs=ones[:],
                                 start=(i == 0), stop=(i == N_WARM - 1))

        # ---- mm1 + swish:  sw[uc] [128, C] bf16 ----
        sws = []
        for uc in range(4):
            hps = bigps.tile([P, C], f32, tag="big", name=f"h{uc}")
            for kc in range(8):
                for (o, TN) in chunks:
                    nc.tensor.matmul(
                        hps[:, o:o + TN],
                        lhsT=w1u[uc][:, kc * P:(kc + 1) * P],
                        rhs=xk[kc][:, o:o + TN],
                        start=(kc == 0), stop=(kc == 7),
                    )
            th = wpool.tile([P, C], f32, tag="th")
            if b1_zero:
                nc.scalar.activation(th[:], hps[:], Tanh, scale=0.5)
            else:
                nc.scalar.activation(th[:], hps[:], Tanh, scale=0.5,
                                     bias=b1sb[:, uc:uc + 1])
            sw = swpool.tile([P, C], bf16, tag="sw", name=f"sw{uc}")
            if b1_zero:
                nc.vector.scalar_tensor_tensor(
                    sw[:], th[:], 1.0, hps[:], op0=add, op1=mult)
            else:
                y = wpool.tile([P, C], f32, tag="y")
                nc.vector.tensor_scalar(
                    y[:], hps[:], b1sb[:, uc:uc + 1], None, op0=add)
                nc.vector.scalar_tensor_tensor(
                    sw[:], th[:], 1.0, y[:], op0=add, op1=mult)
            sws.append(sw)

        # ---- per-vc ----
        def emit_zps(vc):
            zps = bigps.tile([P, C], f32, tag="big", name=f"z{vc}")
            for uc in range(4):
                for (o, TN) in chunks:
                    nc.tensor.matmul(
                        zps[:, o:o + TN],
                        lhsT=puc[uc][:, vc * P:(vc + 1) * P],
                        rhs=sws[uc][:, o:o + TN],
                        start=(uc == 0), stop=(uc == 3),
                    )
            return zps

        def emit_elem(vc, zps):
            t2 = wpool.tile([P, C], f32, tag="t2", name=f"t2_{vc}")
            nc.scalar.activation(t2[:], zps[:], Tanh, scale=0.5)
            g = [None] * 8
            for j in (1, 3):
                g[j] = gpool.tile([P, C], bf16, tag="g", name=f"g{j}_{vc}")
                nc.scalar.activation(g[j][:], t2[:], Exp,
                                     scale=float(esc[j]), bias=bias_of[j])
            # remaining powers as bf16 DVE products (2x mode, no gpsimd to
            # avoid the shared SBUF-port contention)
            for j, (ja, jb) in ((2, (1, 1)), (4, (1, 3)), (5, (2, 3)),
                                (6, (3, 3)), (7, (3, 4))):
                g[j] = gpool.tile([P, C], bf16, tag="g", name=f"g{j}_{vc}")
                nc.vector.tensor_tensor(g[j][:], g[ja][:], g[jb][:], mult)
            s2 = wpool.tile([P, C], f32, tag="s2", name=f"s2_{vc}")
            nc.scalar.activation(s2[:], t2[:], Square, scale=1.0, bias=one_b)
            F = wpool.tile([P, C], f32, tag="F", name=f"F_{vc}")
            nc.scalar.activation(F[:], s2[:], Exp, scale=-8.0)
            return g, F

        # num j-order by g availability: q1, q3 (ACT) then q2 (GP), q4
        # (DVE), q5 (ACT), q6 (GP), q7 (DVE)
        J_ORDER = (1, 3, 2, 4, 5, 6, 7)

        def emit_num_out(vc, g, F):
            # PE: per 512-chunk, 7 accumulating diag matmuls; cv0 folds
            # into the final stt; one output DMA per vc
            ov = wpool.tile([P, C], f32, tag="ov", name=f"ov{vc}")
            for ci, (o, TN) in enumerate(chunks):
                nps = npps.tile([P, 512], f32, tag="np", name=f"n{vc}_{ci}")
                for jn, j in enumerate(J_ORDER):
                    nc.tensor.matmul(
                        nps[:, :TN],
                        lhsT=auxsb[:, vc * 8 + j, :],
                        rhs=g[j][:, o:o + TN],
                        start=(jn == 0), stop=(jn == 6),
                    )
                nc.vector.scalar_tensor_tensor(
                    ov[:, o:o + TN], nps[:, :TN], cv0sb[:, vc:vc + 1],
                    F[:, o:o + TN], op0=add, op1=mult)
            nc.sync.dma_start(outT_r[:, vc, :], ov[:])

        zps_q = {}
        elems = {}
        zps_q[0] = emit_zps(0)
        elems[0] = emit_elem(0, zps_q[0])
        zps_q[1] = emit_zps(1)
        elems[1] = emit_elem(1, zps_q[1])
        emit_num_out(0, *elems[0])
        zps_q[2] = emit_zps(2)
        elems[2] = emit_elem(2, zps_q[2])
        emit_num_out(1, *elems[1])
        zps_q[3] = emit_zps(3)
        elems[3] = emit_elem(3, zps_q[3])
        emit_num_out(2, *elems[2])
        emit_num_out(3, *elems[3])

    nc.compile()
    return nc, chunks


def _get_program(C, mm_mode, b1_zero):
    key = (C, mm_mode, b1_zero, DVE_VC)
    if key not in _prog_cache:
        _prog_cache[key] = build_program(C, b1_zero)
    return _prog_cache[key]


def _route_on_host(x, Wg, bg):
    """Expert assignment, bitwise-matching the reference's fp32 CPU math."""
    import jax
    import jax.numpy as jnp

    cpu = jax.devices("cpu")[0]
    with jax.default_device(cpu):
        logits = jnp.asarray(x) @ jnp.asarray(Wg) + jnp.asarray(bg)
        eid = np.asarray(jnp.argmax(logits, axis=-1))
    return eid


def make_in_maps(x, W1, b1, proj, ctrl, scaling, Wg, bg, mm_mode=None):
    import ml_dtypes

    bf = ml_dtypes.bfloat16

    x = np.asarray(x, dtype=np.float32)
    eid = _route_on_host(x, Wg, bg)
    order = np.argsort(eid, kind="stable")
    counts = np.bincount(eid, minlength=E_EXP)
    starts = np.zeros(E_EXP + 1, dtype=np.int64)
    starts[1:] = np.cumsum(counts)
    C = int(max(counts.max(), 1))
    C = ((C + P - 1) // P) * P

    _, a_j, _ = _basis_consts()

    cvf = (np.asarray(ctrl, np.float32)
           * np.asarray(scaling, np.float32)[:, None, :])  # [E, B, U]
    cvs = cvf * (a_j / THETA)[None, :, None]               # [E, B, U]
    proj5 = 0.5 * np.asarray(proj, np.float32)
    b1f = np.asarray(b1, np.float32)
    b1_zero = not np.any(b1f)

    in_maps = []
    for e in range(E_EXP):
        idx = order[starts[e]:starts[e + 1]]
        xT = np.zeros((D_IN, C), dtype=bf)
        if len(idx):
            xT[:, :len(idx)] = x[idx].T.astype(bf)
        cv_dev = np.ascontiguousarray(
            cvs[e].T.reshape(4, P, B_BAS).transpose(1, 0, 2)).astype(np.float32)
        cv0_dev = np.ascontiguousarray(cv_dev[:, :, 0])
        b1h = np.ascontiguousarray(
            (0.5 * b1f[e]).reshape(4, P).T).astype(np.float32)
        # aux[p, vc*8+j, m] = (m==p) * cvs[e, j, vc*128+p]   (j=0 is cv0)
        aux = np.zeros((P, 32, P), dtype=bf)
        ar = np.arange(P)
        for vc in range(4):
            for j in range(8):
                aux[ar, vc * 8 + j, ar] = cvs[e][j, vc * P:(vc + 1) * P]
        w1h = np.ascontiguousarray(
            np.asarray(W1[e], np.float32).reshape(8, P, 4, P)
            .transpose(2, 1, 0, 3).reshape(4, P, 8 * P)).astype(bf)
        in_maps.append({
            "xT": xT,
            "w1": w1h,
            "p5": proj5[e].astype(bf),
            "aux": aux,
            "cv0": cv0_dev,
            "cvj": cv_dev,
            "b1h": b1h,
        })
    return in_maps, order, starts, counts, C, b1_zero


def kernel(x, W1, b1, proj, ctrl, scaling, Wg, bg):
    from concourse.bass_utils import run_bass_kernel_spmd

    in_maps, order, starts, counts, C, b1_zero = make_in_maps(
        x, W1, b1, proj, ctrl, scaling, Wg, bg, MM_MODE)
    nc, _ = _get_program(C, MM_MODE, b1_zero)

    res = run_bass_kernel_spmd(nc, in_maps, list(range(N_CORES)))

    out = np.empty((N_TOK, U_DIM), dtype=np.float32)
    for e in range(E_EXP):
        cnt = int(counts[e])
        if cnt:
            out[order[starts[e]:starts[e + 1]]] = res.results[e]["outT"][:, :cnt].T
    return out


# revision 37
# speedup vs baseline: 1.0375x; 1.0375x over previous
"""MoE (top-1 routed) Trainium2 kernel — v3.

Routing on host (bitwise-matching the reference's fp32 `x @ Wg + bg`
argmax on CPU); expert e's tokens run on NeuronCore e (expert-parallel,
all-reduce-free).  Device math per core, transposed layout (features on
partitions, tokens on free dim), t2 = tanh(z/2), xn = (1+t2)/2,
k_j = j/7, b_j = exp(-32 (xn-k_j)^2):

    h^T  = W1^T x^T                  (PE bf16, K=1024)
    sw   = (tanh(h/2) + 1) * h       == 2*swish(h)     (ACT+DVE)
    z^T  = (0.5*proj)^T sw           (PE bf16)
    q^j  = exp(esc_j*(t2+1)), esc_j = 32j/7   (odd j=1,3,5 on ACT;
           q2=q1^2, q4=q1*q3, q6=q3^2, q7=q3*q4 as bf16 DVE mults)
    F    = exp(-8 (1+t2)^2) == b_0   (ACT square + exp)
    num  = cv0' + sum_j cvj' q^j     (PE diag-bf16 matmuls, cv0 via a
                                      ones-stream; one vc chunk on DVE)
    out  = F * num                   (gpsimd)

cvj' = ctrl_j*scaling*a_j/theta with a_j = exp(-32 k_j^2).  The RBF
normalizer sum_j b_j is ~= theta = 2.193299 (theta-function flatness,
5.3e-3 worst for observed xn in [0.27,0.77]) and is folded into cvj',
removing the denominator + reciprocal entirely.

All elementwise ops run full-width [128, C] (C~1152) in ONE instruction
to amortize the fixed ~230ns/instr engine overheads: PSUM h/z tiles are
multi-bank mega-tiles; matmuls write 512-aligned in-bank slices.
PSUM budget: 2 mega slots (3 banks each) + 2 single-bank num slots = 8.
"""

import os
from contextlib import ExitStack

import numpy as np

N_TOK, D_IN, U_DIM, E_EXP, B_BAS = 8192, 1024, 512, 8, 8
N_CORES = 8
P = 128
THETA = 2.1932985352029515

MM_MODE = os.environ.get("MOE_MM_MODE", "bf16")
DVE_VC = int(os.environ.get("MOE_DVE_VC", "0"))  # vc chunk whose num runs on DVE

_prog_cache = {}


def _basis_consts():
    ks = np.linspace(0.0, 1.0, B_BAS).astype(np.float64)
    a = np.exp(-32.0 * ks * ks)
    esc = 32.0 * ks * 7.0 / 7.0  # 32*j/7
    return ks, a, esc


def build_program(C, b1_zero):
    import concourse.tile as tile
    from concourse import bacc, mybir

    f32 = mybir.dt.float32
    bf16 = mybir.dt.bfloat16
    add = mybir.AluOpType.add
    mult = mybir.AluOpType.mult
    Tanh = mybir.ActivationFunctionType.Tanh
    Exp = mybir.ActivationFunctionType.Exp
    Square = mybir.ActivationFunctionType.Square

    assert C % 128 == 0
    # 512-wide bank-aligned chunks (the matmul write granularity)
    chunks = []
    t0 = 0
    while t0 < C:
        chunks.append((t0, min(512, C - t0)))
        t0 += 512

    _, _, esc = _basis_consts()

    nc = bacc.Bacc("TRN2", target_bir_lowering=False, debug=False,
                   num_devices=N_CORES)

    xT = nc.dram_tensor("xT", [D_IN, C], bf16, kind="ExternalInput").ap()
    w1 = nc.dram_tensor("w1", [4, P, 8 * P], bf16, kind="ExternalInput").ap()
    p5 = nc.dram_tensor("p5", [U_DIM, U_DIM], bf16, kind="ExternalInput").ap()
    aux = nc.dram_tensor("aux", [P, 32, P], bf16, kind="ExternalInput").ap()
    cv0 = nc.dram_tensor("cv0", [P, 4], f32, kind="ExternalInput").ap()
    cvj = nc.dram_tensor("cvj", [P, 4, B_BAS], f32, kind="ExternalInput").ap()
    b1h = nc.dram_tensor("b1h", [P, 4], f32, kind="ExternalInput").ap()
    outT = nc.dram_tensor("outT", [U_DIM, C], f32, kind="ExternalOutput").ap()

    xT_r = xT.rearrange("(kc p) c -> p kc c", p=P)          # [128, 8, C]
    w1_r = w1.rearrange("u p k -> p u k")                   # [128, 4, 1024]
    p5_r = p5.rearrange("(uc p) v -> p uc v", p=P)          # [128, 4, 512]
    outT_r = outT.rearrange("(vc p) c -> p vc c", p=P)      # [128, 4, C]

    with tile.TileContext(nc) as tc, ExitStack() as ctx:
        cpool = ctx.enter_context(tc.tile_pool(name="consts", bufs=1))
        bigps = ctx.enter_context(tc.tile_pool(name="bigps", bufs=2,
                                               space="PSUM"))
        npps = bigps
        wpool = ctx.enter_context(tc.tile_pool(name="work", bufs=2))
        gpool = ctx.enter_context(tc.tile_pool(name="g", bufs=14))
        swpool = gpool

        # ---- input DMA ----
        w1u = []
        for uc in range(4):
            t = cpool.tile([P, 8 * P], bf16, tag=f"w1_{uc}")
            w1u.append(t)
        xk = [cpool.tile([P, C], bf16, tag=f"x{kc}", name=f"x{kc}")
              for kc in range(8)]
        # x + w1 race in first on sync+scalar, arrival ~ consumption order;
        # late-needed bulk (p5, aux, cvj) queues behind on sync.  The very
        # first 512-col chunk of xk0 and kc0 of w1u0 ship separately so the
        # first matmul can start ~2us earlier.
        # ones memset on the idle DVE queue so the PE warmup starts ASAP
        ones = cpool.tile([P, 512], bf16, tag="ones")
        nc.vector.memset(ones[:], 1.0)
        nc.sync.dma_start(xk[0][:, 0:512], xT_r[:, 0, 0:512])
        nc.scalar.dma_start(w1u[0][:, 0:P], w1_r[:, 0, 0:P])
        nc.sync.dma_start(xk[0][:, 512:C], xT_r[:, 0, 512:C])
        nc.scalar.dma_start(xk[1][:], xT_r[:, 1, :])
        nc.scalar.dma_start(w1u[0][:, P:8 * P], w1_r[:, 0, P:8 * P])
        for kc in (3, 5, 7):
            nc.sync.dma_start(xk[kc][:], xT_r[:, kc, :])
        nc.scalar.dma_start(xk[2][:], xT_r[:, 2, :])
        nc.scalar.dma_start(w1u[1][:], w1_r[:, 1, :])
        nc.scalar.dma_start(xk[4][:], xT_r[:, 4, :])
        nc.scalar.dma_start(xk[6][:], xT_r[:, 6, :])
        nc.scalar.dma_start(w1u[2][:], w1_r[:, 2, :])
        nc.sync.dma_start(w1u[3][:], w1_r[:, 3, :])
        puc = []
        for uc in range(4):
            t = cpool.tile([P, U_DIM], bf16, tag=f"p5_{uc}")
            q = nc.sync if uc % 2 == 0 else nc.scalar
            q.dma_start(t[:], p5_r[:, uc, :])
            puc.append(t)
        cv0sb = cpool.tile([P, 4], f32, tag="cv0")
        nc.sync.dma_start(cv0sb[:], cv0[:])
        cvjsb = cpool.tile([P, 4, B_BAS], f32, tag="cvj")
        nc.sync.dma_start(cvjsb[:], cvj[:])
        auxsb = cpool.tile([P, 32, P], bf16, tag="aux")
        nc.scalar.dma_start(auxsb[:], aux[:])
        if not b1_zero:
            b1sb = cpool.tile([P, 4], f32, tag="b1h")
            nc.sync.dma_start(b1sb[:], b1h[:])
        # bias constants for ACT + bf16 ones (rhs for the cv0 stream)
        bias_vals = [float(esc[1]), float(esc[3]), float(esc[5]), 1.0]
        bsb = cpool.tile([P, len(bias_vals)], f32, tag="bias")
        for i, v in enumerate(bias_vals):
            nc.gpsimd.memset(bsb[:, i:i + 1], v)
        bias_of = {1: bsb[:, 0:1], 3: bsb[:, 1:2], 5: bsb[:, 2:3]}
        one_b = bsb[:, 3:4]

        # ---- PE warmup: ramp the tensor-engine p-state to max while the
        # first x/w1 DMAs are in flight (dummy matmuls on the ones tile) ----
        N_WARM = int(os.environ.get("MOE_WARM", "16"))
        if N_WARM:
            wps = npps.tile([P, 512], f32, tag="np", name="warm")
            for i in range(N_WARM):
                nc.tensor.matmul(wps[:], lhsT=ones[:, 0:P], rhs=ones[:],
                                 start=(i == 0), stop=(i == N_WARM - 1))

        # ---- mm1 + swish:  sw[uc] [128, C] bf16 ----
        sws = []
        for uc in range(4):
            hps = bigps.tile([P, C], f32, tag="big", name=f"h{uc}")
            for kc in range(8):
                for (o, TN) in chunks:
                    nc.tensor.matmul(
                        hps[:, o:o + TN],
                        lhsT=w1u[uc][:, kc * P:(kc + 1) * P],
                        rhs=xk[kc][:, o:o + TN],
                        start=(kc == 0), stop=(kc == 7),
                    )
            th = wpool.tile([P, C], f32, tag="th")
            if b1_zero:
                nc.scalar.activation(th[:], hps[:], Tanh, scale=0.5)
            else:
                nc.scalar.activation(th[:], hps[:], Tanh, scale=0.5,
                                     bias=b1sb[:, uc:uc + 1])
            sw = swpool.tile([P, C], bf16, tag="sw", name=f"sw{uc}")
            if b1_zero:
                nc.vector.scalar_tensor_tensor(
                    sw[:], th[:], 1.0, hps[:], op0=add, op1=mult)
            else:
                y = wpool.tile([P, C], f32, tag="y")
                nc.vector.tensor_scalar(
                    y[:], hps[:], b1sb[:, uc:uc + 1], None, op0=add)
                nc.vector.scalar_tensor_tensor(
                    sw[:], th[:], 1.0, y[:], op0=add, op1=mult)
            sws.append(sw)

        # ---- per-vc ----
        def emit_zps(vc):
            zps = bigps.tile([P, C], f32, tag="big", name=f"z{vc}")
            for uc in range(4):
                for (o, TN) in chunks:
                    nc.tensor.matmul(
                        zps[:, o:o + TN],
                        lhsT=puc[uc][:, vc * P:(vc + 1) * P],
                        rhs=sws[uc][:, o:o + TN],
                        start=(uc == 0), stop=(uc == 3),
                    )
            return zps

        def emit_elem(vc, zps):
            t2 = wpool.tile([P, C], f32, tag="t2", name=f"t2_{vc}")
            nc.scalar.activation(t2[:], zps[:], Tanh, scale=0.5)
            g = [None] * 8
            for j in (1, 3):
                g[j] = gpool.tile([P, C], bf16, tag="g", name=f"g{j}_{vc}")
                nc.scalar.activation(g[j][:], t2[:], Exp,
                                     scale=float(esc[j]), bias=bias_of[j])
            # remaining powers as bf16 DVE products (2x mode, no gpsimd to
            # avoid the shared SBUF-port contention)
            for j, (ja, jb) in ((2, (1, 1)), (4, (1, 3)), (5, (2, 3)),
                                (6, (3, 3)), (7, (3, 4))):
                g[j] = gpool.tile([P, C], bf16, tag="g", name=f"g{j}_{vc}")
                nc.vector.tensor_tensor(g[j][:], g[ja][:], g[jb][:], mult)
            s2 = wpool.tile([P, C], f32, tag="s2", name=f"s2_{vc}")
            nc.scalar.activation(s2[:], t2[:], Square, scale=1.0, bias=one_b)
            F = wpool.tile([P, C], f32, tag="F", name=f"F_{vc}")
            nc.scalar.activation(F[:], s2[:], Exp, scale=-8.0)
            return g, F

        # num j-order by g availability: q1, q3 (ACT) then q2 (GP), q4
        # (DVE), q5 (ACT), q6 (GP), q7 (DVE)
        J_ORDER = (1, 3, 2, 4, 5, 6, 7)

        def emit_num_out(vc, g, F):
            # PE: per 512-chunk, 7 accumulating diag matmuls; cv0 folds
            # into the final stt; one output DMA per vc
            ov = wpool.tile([P, C], f32, tag="ov", name=f"ov{vc}")
            for ci, (o, TN) in enumerate(chunks):
                nps = npps.tile([P, 512], f32, tag="np", name=f"n{vc}_{ci}")
                for jn, j in enumerate(J_ORDER):
                    nc.tensor.matmul(
                        nps[:, :TN],
                        lhsT=auxsb[:, vc * 8 + j, :],
                        rhs=g[j][:, o:o + TN],
                        start=(jn == 0), stop=(jn == 6),
                    )
                nc.vector.scalar_tensor_tensor(
                    ov[:, o:o + TN], nps[:, :TN], cv0sb[:, vc:vc + 1],
                    F[:, o:o + TN], op0=add, op1=mult)
            nc.sync.dma_start(outT_r[:, vc, :], ov[:])

        zps_q = {}
        elems = {}
        zps_q[0] = emit_zps(0)
        elems[0] = emit_elem(0, zps_q[0])
        zps_q[1] = emit_zps(1)
        elems[1] = emit_elem(1, zps_q[1])
        emit_num_out(0, *elems[0])
        zps_q[2] = emit_zps(2)
        elems[2] = emit_elem(2, zps_q[2])
        emit_num_out(1, *elems[1])
        zps_q[3] = emit_zps(3)
        elems[3] = emit_elem(3, zps_q[3])
        emit_num_out(2, *elems[2])
        emit_num_out(3, *elems[3])

    nc.compile()
    return nc, chunks


def _get_program(C, mm_mode, b1_zero):
    key = (C, mm_mode, b1_zero, DVE_VC)
    if key not in _prog_cache:
        _prog_cache[key] = build_program(C, b1_zero)
    return _prog_cache[key]


def _route_on_host(x, Wg, bg):
    """Expert assignment, bitwise-matching the reference's fp32 CPU math."""
    import jax
    import jax.numpy as jnp

    cpu = jax.devices("cpu")[0]
    with jax.default_device(cpu):
        logits = jnp.asarray(x) @ jnp.asarray(Wg) + jnp.asarray(bg)
        eid = np.asarray(jnp.argmax(logits, axis=-1))
    return eid


def make_in_maps(x, W1, b1, proj, ctrl, scaling, Wg, bg, mm_mode=None):
    import ml_dtypes

    bf = ml_dtypes.bfloat16

    x = np.asarray(x, dtype=np.float32)
    eid = _route_on_host(x, Wg, bg)
    order = np.argsort(eid, kind="stable")
    counts = np.bincount(eid, minlength=E_EXP)
    starts = np.zeros(E_EXP + 1, dtype=np.int64)
    starts[1:] = np.cumsum(counts)
    C = int(max(counts.max(), 1))
    C = ((C + P - 1) // P) * P

    _, a_j, _ = _basis_consts()

    cvf = (np.asarray(ctrl, np.float32)
           * np.asarray(scaling, np.float32)[:, None, :])  # [E, B, U]
    cvs = cvf * (a_j / THETA)[None, :, None]               # [E, B, U]
    proj5 = 0.5 * np.asarray(proj, np.float32)
    b1f = np.asarray(b1, np.float32)
    b1_zero = not np.any(b1f)

    in_maps = []
    for e in range(E_EXP):
        idx = order[starts[e]:starts[e + 1]]
        xT = np.zeros((D_IN, C), dtype=bf)
        if len(idx):
            xT[:, :len(idx)] = x[idx].T.astype(bf)
        cv_dev = np.ascontiguousarray(
            cvs[e].T.reshape(4, P, B_BAS).transpose(1, 0, 2)).astype(np.float32)
        cv0_dev = np.ascontiguousarray(cv_dev[:, :, 0])
        b1h = np.ascontiguousarray(
            (0.5 * b1f[e]).reshape(4, P).T).astype(np.float32)
        # aux[p, vc*8+j, m] = (m==p) * cvs[e, j, vc*128+p]   (j=0 is cv0)
        aux = np.zeros((P, 32, P), dtype=bf)
        ar = np.arange(P)
        for vc in range(4):
            for j in range(8):
                aux[ar, vc * 8 + j, ar] = cvs[e][j, vc * P:(vc + 1) * P]
        w1h = np.ascontiguousarray(
            np.asarray(W1[e], np.float32).reshape(8, P, 4, P)
            .transpose(2, 1, 0, 3).reshape(4, P, 8 * P)).astype(bf)
        in_maps.append({
            "xT": xT,
            "w1": w1h,
            "p5": proj5[e].astype(bf),
            "aux": aux,
            "cv0": cv0_dev,
            "cvj": cv_dev,
            "b1h": b1h,
        })
    return in_maps, order, starts, counts, C, b1_zero


def kernel(x, W1, b1, proj, ctrl, scaling, Wg, bg):
    from concourse.bass_utils import run_bass_kernel_spmd

    in_maps, order, starts, counts, C, b1_zero = make_in_maps(
        x, W1, b1, proj, ctrl, scaling, Wg, bg, MM_MODE)
    nc, _ = _get_program(C, MM_MODE, b1_zero)

    res = run_bass_kernel_spmd(nc, in_maps, list(range(N_CORES)))

    out = np.empty((N_TOK, U_DIM), dtype=np.float32)
    for e in range(E_EXP):
        cnt = int(counts[e])
        if cnt:
            out[order[starts[e]:starts[e + 1]]] = res.results[e]["outT"][:, :cnt].T
    return out
